# revision 12
# baseline (speedup 1.0000x reference)
"""Trainium2 Bass kernel for nn_Diffusion_59760174956877 (gnn_message_passing).

Us[t] = sum_{l,r,e} atn[l,r,e] * exp(-((dist[t,l,r]-mu_e)/sigma)^2)
  atn[l,r,e] = sum_f lig_feat[l,e,f] * rec_feat[r,e,f]

Sharding: R (1024 receptor atoms) split across 8 cores, 128 each. Every core
computes all T=16 transforms on its receptor slice; host sums the 8 partial
energy vectors.

Per-core layout: partitions p = l'*32 + e (4 ligand atoms x 32 RBF centers),
free = (t, r). The ACT engine evaluates exp(-((d-mu)/sigma)^2) in one pass via
Derivative_Erf with per-partition bias (-mu_e/sigma); PE broadcasts d into that
layout with a block-one-hot matmul and also performs the sum over partitions
(l', e) with an accumulating ones-matmul; DVE does the single rbf*atn product
pass in fp16.
"""
import sys
sys.path.insert(0, "/opt/trn_rl_repo")
import numpy as np

L, R, T, E, F = 128, 1024, 16, 32, 64
NC = 8
RS = R // NC  # 128 receptors per core
SIGMA = 0.3125           # |(RBF_START - RBF_END)/RBF_STEPS|
INV_SIGMA = 1.0 / SIGMA
MU = np.linspace(0.0, 10.0, E, dtype=np.float64)
SQRT_PI_OVER_2 = float(np.sqrt(np.pi) / 2.0)
TH = T // 2  # t-half size (8)

_cached = None


def _build():
    global _cached
    if _cached is not None:
        return _cached

    import concourse.bass as bass
    import concourse.bacc as bacc
    import concourse.tile as tile
    from concourse import mybir

    f32 = mybir.dt.float32
    f16 = mybir.dt.float16
    bf16 = mybir.dt.bfloat16
    f32r = mybir.dt.float32r

    nc = bacc.Bacc("TRN2", target_bir_lowering=False, debug=False, num_devices=NC)

    ligT_in = nc.dram_tensor("ligT_in", [F, E * L], f16, kind="ExternalInput").ap()
    recT_in = nc.dram_tensor("recT_in", [F, E * RS], f16, kind="ExternalInput").ap()
    ligc_in = nc.dram_tensor("ligc_in", [L, T * 3], f32, kind="ExternalInput").ap()
    recc_in = nc.dram_tensor("recc_in", [128, 3 * RS], f32, kind="ExternalInput").ap()
    bias_in = nc.dram_tensor("bias_in", [128, 1], f32, kind="ExternalInput").ap()
    sel_in = nc.dram_tensor("sel_in", [128, 32 * 128], f32r, kind="ExternalInput").ap()
    ones_in = nc.dram_tensor("ones_in", [128, 1], f16, kind="ExternalInput").ap()
    us_out = nc.dram_tensor("us_out", [1, T], f32, kind="ExternalOutput").ap()

    bounce = nc.dram_tensor("atn_bounce", [L, E * RS], f16)

    with tile.TileContext(nc) as tc:
        with tc.tile_pool(name="const", bufs=1) as cp:
            # spread loads across DMA queues; critical path first:
            # coords (gate DVE d^2) tiny -> sync first; ligT/recT gate PE
            t_ligc = cp.tile([L, T * 3], f32)
            nc.sync.dma_start(out=t_ligc, in_=ligc_in)
            t_recc = cp.tile([128, 3 * RS], f32)
            nc.sync.dma_start(out=t_recc, in_=recc_in)
            t_bias = cp.tile([128, 1], f32)
            nc.sync.dma_start(out=t_bias, in_=bias_in)
            t_ligT = cp.tile([F, E * L], f16)
            nc.sync.dma_start(out=t_ligT, in_=ligT_in)
            t_recT = cp.tile([F, E * RS], f16)
            nc.scalar.dma_start(out=t_recT, in_=recT_in)
            t_ones = cp.tile([128, 1], f16)
            nc.sync.dma_start(out=t_ones, in_=ones_in)
            t_sel = cp.tile([128, 32 * 128], f32r)
            nc.scalar.dma_start(out=t_sel, in_=sel_in)

            t_atn = cp.tile([L, E * RS], f16)      # atn * sqrt(pi)/2, [l, (e, r)]
            t_atnT = cp.tile([128, 32 * RS], f16)  # [p=(l',e), (g, r)]
            t_d2 = cp.tile([128, T * RS], f32)     # [l, (t, r)]
            t_d = cp.tile([128, T * RS], f32r)
            t_final = cp.tile([1, T], f32)

            # ---- Phase 1: attention coefficients via 32 per-e matmuls
            with tc.tile_pool(name="psA", bufs=1, space="PSUM") as psA:
                p_atn = psA.tile([L, E * RS], f32)  # all 8 banks
                for e in range(E):
                    sl = slice(e * RS, (e + 1) * RS)
                    nc.tensor.matmul(
                        p_atn[:, sl], t_ligT[:, e * L:(e + 1) * L], t_recT[:, sl],
                        start=True, stop=True,
                    )
                # cast fp32 psum -> fp16 sbuf, folding the sqrt(pi)/2 factor
                nc.scalar.mul(t_atn, p_atn, SQRT_PI_OVER_2)

            # transpose via DRAM bounce into p = l'*32 + e layout
            bnc = bounce.ap()
            nc.sync.dma_start(out=bnc, in_=t_atn)
            src = bnc.rearrange("(g lp) (e r) -> lp e g r", lp=4, e=E)
            for lp in range(4):
                dst_v = t_atnT[lp * 32:(lp + 1) * 32, :].rearrange(
                    "e (g r) -> e g r", g=32)
                eng = nc.sync if lp % 2 == 0 else nc.scalar
                eng.dma_start(out=dst_v, in_=src[lp])

            with (
                tc.tile_pool(name="gp", bufs=3) as gp_pool,
                tc.tile_pool(name="rbf", bufs=3) as rbf_pool,
                tc.tile_pool(name="prod", bufs=3) as prod_pool,
                tc.tile_pool(name="red", bufs=2) as red_pool,
                tc.tile_pool(name="psB", bufs=2, space="PSUM") as psB,
                tc.tile_pool(name="psC", bufs=2, space="PSUM") as psC,
            ):
                for h in range(2):
                    # ---- Phase 2: distances for this t-half (DVE + ACT sqrt)
                    for t in range(h * TH, (h + 1) * TH):
                        scr = gp_pool.tile([128, 3 * RS], f32, tag="scr")
                        for c in range(3):
                            nc.vector.tensor_scalar(
                                out=scr[:, c * RS:(c + 1) * RS],
                                in0=t_recc[:, c * RS:(c + 1) * RS],
                                scalar1=t_ligc[:, t * 3 + c:t * 3 + c + 1],
                                scalar2=None,
                                op0=mybir.AluOpType.subtract,
                            )
                        sq = gp_pool.tile([128, 3 * RS], f32, tag="sq")
                        nc.vector.tensor_tensor(
                            out=sq, in0=scr, in1=scr, op=mybir.AluOpType.mult)
                        nc.vector.tensor_tensor(
                            out=sq[:, 0:RS], in0=sq[:, 0:RS], in1=sq[:, RS:2 * RS],
                            op=mybir.AluOpType.add)
                        nc.vector.tensor_tensor(
                            out=t_d2[:, t * RS:(t + 1) * RS], in0=sq[:, 0:RS],
                            in1=sq[:, 2 * RS:3 * RS], op=mybir.AluOpType.add)
                    hs = slice(h * TH * RS, (h + 1) * TH * RS)
                    nc.scalar.sqrt(t_d[:, hs], t_d2[:, hs])

                    # ---- Phase 3: main loop over ligand groups
                    p_us = psC.tile([1, TH * RS], f32)
                    for g in range(32):
                        p_bc = psB.tile([128, TH * RS], f32)
                        lhs_sel = t_sel[:, g * 128:(g + 1) * 128]
                        d_rows = t_d[:, hs]
                        nc.tensor.matmul(
                            p_bc[:, 0:512], lhs_sel, d_rows[:, 0:512],
                            start=True, stop=True)
                        nc.tensor.matmul(
                            p_bc[:, 512:1024], lhs_sel, d_rows[:, 512:1024],
                            start=True, stop=True)
                        t_rbf = rbf_pool.tile([128, TH * RS], f16)
                        nc.scalar.activation(
                            t_rbf, p_bc, mybir.ActivationFunctionType.Derivative_Erf,
                            bias=t_bias[:, 0:1], scale=INV_SIGMA,
                        )
                        t_prod = prod_pool.tile([128, TH * RS], f16)
                        atn_b = t_atnT[:, g * RS:(g + 1) * RS].unsqueeze(1).broadcast_to(
                            [128, TH, RS])
                        nc.vector.tensor_tensor(
                            out=t_prod.rearrange("p (t r) -> p t r", t=TH),
                            in0=t_rbf.rearrange("p (t r) -> p t r", t=TH),
                            in1=atn_b, op=mybir.AluOpType.mult,
                        )
                        nc.tensor.matmul(
                            p_us[0:1, 0:512], t_ones, t_prod[:, 0:512],
                            start=(g == 0), stop=(g == 31))
                        nc.tensor.matmul(
                            p_us[0:1, 512:1024], t_ones, t_prod[:, 512:1024],
                            start=(g == 0), stop=(g == 31))

                    t_us = red_pool.tile([1, TH * RS], f32, tag="uscopy")
                    nc.vector.tensor_copy(t_us, p_us)
                    nc.vector.tensor_reduce(
                        out=t_final[:, h * TH:(h + 1) * TH],
                        in_=t_us.rearrange("o (t r) -> o t r", t=TH),
                        axis=mybir.AxisListType.X, op=mybir.AluOpType.add,
                    )

            nc.sync.dma_start(out=us_out, in_=t_final)

    nc.compile()
    _cached = nc
    return nc


def _prep_inputs(lig_feat, rec_feat, lig_coords, rec_coords):
    lig_feat = np.asarray(lig_feat, dtype=np.float32)
    rec_feat = np.asarray(rec_feat, dtype=np.float32)
    lig_coords = np.asarray(lig_coords, dtype=np.float32)
    rec_coords = np.asarray(rec_coords, dtype=np.float32)

    ligT = np.ascontiguousarray(
        lig_feat.transpose(2, 1, 0).reshape(F, E * L)).astype(np.float16)
    ligc = np.ascontiguousarray(
        lig_coords.transpose(1, 0, 2).reshape(L, T * 3)).astype(np.float32)
    bias = (np.tile(MU, 4) * (-INV_SIGMA)).reshape(128, 1).astype(np.float32)
    # sel[k, m*128+p] = 1 iff k == 4m + p//32  (selects ligand rows 4g..4g+3
    # out of a 32-row block, g = (g//8)*8 + m)
    # sel[k, g*128+p] = 1 iff k == 4g + p//32: out[p] = d[4g + p//32]
    sel = np.zeros((128, 32 * 128), dtype=np.float32)
    for g in range(32):
        for p in range(128):
            sel[4 * g + p // 32, g * 128 + p] = 1.0

    ones = np.ones((128, 1), dtype=np.float16)

    in_maps = []
    for c in range(NC):
        sl = slice(c * RS, (c + 1) * RS)
        recT = np.ascontiguousarray(
            rec_feat[sl].transpose(2, 1, 0).reshape(F, E * RS)).astype(np.float16)
        recc = np.tile(
            np.ascontiguousarray(rec_coords[sl].T.reshape(1, 3 * RS)), (128, 1)
        ).astype(np.float32)
        in_maps.append({
            "ligT_in": ligT, "recT_in": recT, "ligc_in": ligc, "recc_in": recc,
            "bias_in": bias, "sel_in": sel, "ones_in": ones,
        })
    return in_maps


def kernel(lig_feat, rec_feat, lig_coords, rec_coords, trace=False, **trace_kw):
    from concourse.bass_utils import run_bass_kernel_spmd

    nc = _build()
    in_maps = _prep_inputs(lig_feat, rec_feat, lig_coords, rec_coords)
    res = run_bass_kernel_spmd(
        nc, in_maps, core_ids=list(range(NC)), trace=trace, **trace_kw)
    us = np.zeros(T, dtype=np.float64)
    for c in range(NC):
        us += res.results[c]["us_out"][0].astype(np.float64)
    out = us.astype(np.float32)
    if trace:
        return out, res
    return out
